# revision 15
# baseline (speedup 1.0000x reference)
"""DiffGraphTransformerSBM Trainium2 kernel (v2).

Data-parallel over batch across 8 NeuronCores (4 graphs per core), with a
fully transpose-free feature-major layout on each core:

  - activations kept feature-major: h[f, t] (features on partitions)
  - scores computed directly transposed  sT[k, q] = kT.T @ qT, with the two
    heads of a partition-group issued back-to-back so they run concurrently
    on disjoint PE row-groups (auto tile_position from base_partition 0/64)
  - softmax without max-subtraction (scores are O(1e-1) by construction)
  - GraphiT renormalization folded algebraically:
        a = softmax(s)*pe / (sum(softmax(s)*pe) + 1e-6)
          = e*pe' / sum_k(e*pe'),  e = exp(s), pe' = pe + 1e-6
    (numerator pe vs pe' differs by O(1e-6) absolute in a — negligible)
  - o-matmuls col-tiled: the two heads of a group write partitions 0-63 /
    64-127 of ONE psum bank concurrently; denominators are separate M=1
    col-tiled matmuls accumulated into a shared "den" bank
  - LayerNorm stats via col-tiled concurrent rank-reduced matmuls, row math
    on [1,N] rows, rank-1 bf16 broadcast matmuls, quad-batched DVE apply

PSUM budget (8 banks): qk 2 (bufs=2) + s 2 (bufs=1 of [128,2N]) +
o 1 + den 1 + lnb 2 (bufs=1 of [128,2,N]).

Self-contained: hardcodes all shapes; only imports the concourse toolchain.
"""

import os
import sys

import numpy as np
import ml_dtypes

for _p in ("/opt/trn_rl_repo", "/root/.axon_site/_ro/trn_rl_repo"):
    if os.path.isdir(_p) and _p not in sys.path:
        sys.path.append(_p)

import concourse.bass as bass
import concourse.bacc as bacc
import concourse.mybir as mybir
import concourse.tile as tile
from concourse import bass_utils

F32 = mybir.dt.float32
BF16 = mybir.dt.bfloat16
AF = mybir.ActivationFunctionType
ALU = mybir.AluOpType
bf = ml_dtypes.bfloat16

# problem dims
B, N, IN, D, H, DH, DFF, L, C = 32, 512, 64, 512, 8, 64, 2048, 4, 6
NCORES = 8
G = B // NCORES          # graphs per core
FC = D // 128            # feature chunks
KC = N // 128            # key-token chunks per graph
FFC = DFF // 128         # ffn chunks
T = G * N                # tokens per core


def _build(flags):
    """Build the per-core Bass program. flags: dict of which biases/affines exist."""
    nc = bacc.Bacc("TRN2", target_bir_lowering=False, debug=False, enable_asserts=False)

    # ---- DRAM I/O ----
    d_xT = nc.dram_tensor("xT", [G, IN, N], BF16, kind="ExternalInput").ap()
    d_peT = nc.dram_tensor("peT", [G, N, N], BF16, kind="ExternalInput").ap()
    d_deg = nc.dram_tensor("deg", [G, N], F32, kind="ExternalInput").ap()
    d_wembT = nc.dram_tensor("wembT", [IN, D], BF16, kind="ExternalInput").ap()
    d_qkvT = nc.dram_tensor("qkvT", [L, D, 3 * D], BF16, kind="ExternalInput").ap()
    d_owT = nc.dram_tensor("owT", [L, D, D], BF16, kind="ExternalInput").ap()
    d_l1T = nc.dram_tensor("l1T", [L, D, DFF], BF16, kind="ExternalInput").ap()
    d_l2T = nc.dram_tensor("l2T", [L, DFF, D], BF16, kind="ExternalInput").ap()
    d_c1T = nc.dram_tensor("c1T", [D, D], BF16, kind="ExternalInput").ap()
    d_c2T = nc.dram_tensor("c2T", [D, C], BF16, kind="ExternalInput").ap()
    nbias = flags["nbias_cols"]
    d_bias = None
    if nbias:
        d_bias = nc.dram_tensor("biasblob", [128, nbias], F32, kind="ExternalInput").ap()
    d_vb = None
    if flags["vb"]:
        d_vb = nc.dram_tensor("vbias", [L, 1, D], BF16, kind="ExternalInput").ap()
    d_c2b = None
    if flags["c2b"]:
        d_c2b = nc.dram_tensor("c2bias", [1, C], BF16, kind="ExternalInput").ap()
    # output transposed per graph: [G, C, N]; host transposes back
    d_out = nc.dram_tensor("out", [G, C, N], F32, kind="ExternalOutput").ap()

    with tile.TileContext(nc) as tc:
        with tc.tile_pool(name="persist", bufs=1) as pp, \
             tc.tile_pool(name="wq", bufs=2) as wq, \
             tc.tile_pool(name="wo", bufs=2) as wo, \
             tc.tile_pool(name="w1", bufs=1) as w1p, \
             tc.tile_pool(name="w2", bufs=1) as w2p, \
             tc.tile_pool(name="work", bufs=1) as wk, \
             tc.tile_pool(name="ps", bufs=1, space="PSUM") as ps:

            # ---- persistent loads (order matters: small/early-needed first) ----
            wemb_sb = pp.tile([IN, D], BF16)
            nc.sync.dma_start(out=wemb_sb, in_=d_wembT)
            xT_sb = pp.tile([IN, G, N], BF16)
            nc.sync.dma_start(out=xT_sb, in_=d_xT.rearrange("g i q -> i g q"))
            hg = [pp.tile([128, FC, N], BF16, name=f"h{g}") for g in range(G)]
            degB = pp.tile([128, G, N], F32)
            for g in range(G):
                src = d_deg[g:g + 1, :]
                src_bc = bass.AP(tensor=src.tensor, offset=src.offset,
                                 ap=[[0, 128]] + list(src.ap[1:]))
                nc.sync.dma_start(out=degB[:, g, :], in_=src_bc)
            pe_sb = pp.tile([128, G * KC, N], BF16)
            for g in range(G):
                nc.sync.dma_start(
                    out=pe_sb[:, g * KC:(g + 1) * KC, :],
                    in_=d_peT[g].rearrange("(kc p) q -> p kc q", p=128))
            c1T_sb = pp.tile([128, FC, D], BF16)
            nc.sync.dma_start(out=c1T_sb, in_=d_c1T.rearrange("(kc p) f -> p kc f", p=128))
            c2T_sb = pp.tile([128, FC, C], BF16)
            nc.sync.dma_start(out=c2T_sb, in_=d_c2T.rearrange("(kc p) f -> p kc f", p=128))

            ones64 = pp.tile([128, 64], BF16)
            nc.vector.memset(ones64, 1.0)
            ones128r = pp.tile([1, 128], BF16)
            nc.vector.memset(ones128r, 1.0)
            onescol = pp.tile([128, 1], BF16)
            nc.vector.memset(onescol, 1.0)
            invn = pp.tile([128, 1], BF16)
            nc.vector.memset(invn, 1.0 / D)

            bias_sb = None
            if nbias:
                bias_sb = pp.tile([128, nbias], F32)
                nc.sync.dma_start(out=bias_sb, in_=d_bias)
            vb_sb = None
            if flags["vb"]:
                vb_sb = pp.tile([L, 1, D], BF16)
                nc.sync.dma_start(out=vb_sb, in_=d_vb)
            c2b_sb = None
            if flags["c2b"]:
                c2b_sb = pp.tile([1, C], BF16)
                nc.sync.dma_start(out=c2b_sb, in_=d_c2b)

            bcol = flags["bias_cols"]  # dict name -> start col in bias blob

            def bias_ap(name, l, idx):
                w = {"qkvb": 8, "ob": 4, "l1b": 16, "l2b": 4, "ln1s": 4,
                     "ln1b": 4, "ln2s": 4, "ln2b": 4, "c1b": 0}[name]
                c0 = bcol[name] + l * w + idx
                return bias_sb[:, c0:c0 + 1]

            # ---- embedding: h0[f, t] = W_emb.T.T @ xT ----
            for g in range(G):
                for fc in range(FC):
                    e_ps = ps.tile([128, N], F32, tag="qkv", bufs=1, name="e_ps")
                    nc.tensor.matmul(e_ps, wemb_sb[:, fc * 128:(fc + 1) * 128],
                                     xT_sb[:, g, :], start=True, stop=True)
                    nc.scalar.activation(hg[g][:, fc, :], e_ps, AF.Copy)

            # ---- transformer layers ----
            for l in range(L):
                qkv_t = wq.tile([128, KC, 3 * D], BF16, name="qkv_t")
                nc.sync.dma_start(out=qkv_t, in_=d_qkvT[l].rearrange("(kc p) f -> p kc f", p=128))
                ow_t = wo.tile([128, KC, D], BF16, name="ow_t")
                nc.sync.dma_start(out=ow_t, in_=d_owT[l].rearrange("(kc p) f -> p kc f", p=128))
                l1_t = w1p.tile([128, KC, DFF], BF16, name="l1_t")
                nc.sync.dma_start(out=l1_t, in_=d_l1T[l].rearrange("(kc p) f -> p kc f", p=128))
                l2_t = w2p.tile([128, FFC, D], BF16, name="l2_t")
                nc.sync.dma_start(out=l2_t, in_=d_l2T[l].rearrange("(kc p) f -> p kc f", p=128))

                for g in range(G):
                    hgt = hg[g]

                    # ---- QKV ----
                    qT = wk.tile([128, FC, N], BF16, bufs=2, name="qT")
                    kT = wk.tile([128, FC, N], BF16, bufs=2, name="kT")
                    for fc in range(2 * FC):  # 0..3 q, 4..7 k
                        qk_ps = ps.tile([128, N], F32, tag="qkv", bufs=1, name="qk_ps")
                        for kc in range(KC):
                            nc.tensor.matmul(qk_ps, qkv_t[:, kc, fc * 128:(fc + 1) * 128],
                                             hgt[:, kc, :],
                                             start=(kc == 0), stop=(kc == KC - 1))
                        dst = qT[:, fc, :] if fc < FC else kT[:, fc - FC, :]
                        if flags["qkvb"]:
                            nc.scalar.activation(dst, qk_ps, AF.Copy,
                                                 bias=bias_ap("qkvb", l, fc))
                        else:
                            nc.scalar.activation(dst, qk_ps, AF.Copy)

                    # ---- V (plain layout: v_sb[k-token, d]; heads contiguous) ----
                    v_sb = wk.tile([128, KC, D], BF16, bufs=2, name="v_sb")
                    for tc4 in range(KC):
                        v_ps = ps.tile([128, N], F32, tag="qkv", bufs=1, name="v_ps")
                        for kc in range(KC):
                            nc.tensor.matmul(v_ps, hgt[:, kc, tc4 * 128:(tc4 + 1) * 128],
                                             qkv_t[:, kc, 2 * D:3 * D],
                                             start=(kc == 0), stop=(kc == KC - 1 and not flags["vb"]))
                        if flags["vb"]:
                            nc.tensor.matmul(v_ps, ones128r, vb_sb[l], start=False, stop=True)
                        nc.scalar.activation(v_sb[:, tc4, :], v_ps, AF.Copy)

                    # ---- attention: per grp (2 heads), per kc chunk ----
                    ou = wk.tile([128, FC, N], BF16, bufs=2, name="ou")
                    for grp in range(4):
                        dps = ps.tile([128, N], F32, tag="den", bufs=1, name="den_ps")
                        o_ps = ps.tile([128, N], F32, tag="o", bufs=1, name="o_ps")
                        for kc in range(KC):
                            s_ps = ps.tile([128, 2 * N], F32, tag="s", bufs=1, name="s_ps")
                            for j in range(2):
                                hh = grp * 2 + j
                                po = (hh % 2) * 64
                                pt = hh // 2
                                nc.tensor.matmul(
                                    s_ps[:, j * N:(j + 1) * N],
                                    kT[po:po + 64, pt, kc * 128:(kc + 1) * 128],
                                    qT[po:po + 64, pt, :], start=True, stop=True)
                            em_t = wk.tile([128, 2, N], BF16, bufs=3, name="em_t")
                            nc.scalar.activation(em_t.rearrange("p h q -> p (h q)"),
                                                 s_ps, AF.Exp)
                            pe_slice = pe_sb[:, g * KC + kc, :]
                            pe_bc = bass.AP(tensor=pe_slice.tensor, offset=pe_slice.offset,
                                            ap=[pe_slice.ap[0], [0, 2]] + list(pe_slice.ap[1:]))
                            nc.vector.tensor_mul(em_t, em_t, pe_bc)
                            # o: two heads col-tiled into one bank (partitions 0-63 / 64-127)
                            nc.tensor.matmul(o_ps[0:64, :],
                                             v_sb[:, kc, grp * 128:grp * 128 + 64],
                                             em_t[:, 0, :],
                                             start=(kc == 0), stop=(kc == KC - 1),
                                             tile_position=(0, 0))
                            nc.tensor.matmul(o_ps[64:128, :],
                                             v_sb[:, kc, grp * 128 + 64:grp * 128 + 128],
                                             em_t[:, 1, :],
                                             start=(kc == 0), stop=(kc == KC - 1),
                                             tile_position=(0, 64))
                            # denominators: M=1 col-tiled rows at partitions {0,32}
                            for j in range(2):
                                nc.tensor.matmul(dps[32 * j:32 * j + 1, :],
                                                 onescol, em_t[:, j, :],
                                                 start=(kc == 0), stop=(kc == KC - 1),
                                                 tile_position=(0, 32 * j))
                        # rec rows: deg / den (bf16) on rows {0,32}
                        rec_t = wk.tile([128, N], F32, bufs=1, name="rec_t")
                        nc.vector.reciprocal_approx_fast(out=rec_t[0:33, :],
                                                         in_=dps[0:33, :])
                        rec_bf = wk.tile([128, N], BF16, bufs=1, name="rec_bf")
                        nc.vector.tensor_mul(rec_bf[0:33, :], rec_t[0:33, :],
                                             degB[0:33, g, :])
                        # broadcast rec rows to [128, N] and normalize o
                        # (one PSUM input max per DVE op -> stage db in SBUF)
                        db_ps = ps.tile([128, N], F32, tag="qkv", bufs=1, name="db_ps")
                        for j in range(2):
                            nc.tensor.matmul(db_ps[64 * j:64 * (j + 1), :],
                                             ones64[32 * j:32 * j + 1, :],
                                             rec_bf[32 * j:32 * j + 1, :],
                                             start=True, stop=True,
                                             tile_position=(32 * j, 64 * j))
                        db_sb = wk.tile([128, N], BF16, bufs=2, name="db_sb")
                        nc.scalar.activation(db_sb, db_ps, AF.Copy)
                        nc.vector.tensor_mul(ou[:, grp, :], o_ps, db_sb)

                    # ---- out-proj + residual -> h_sb (pre-LN u) ----
                    for fc in range(FC):
                        op_ps = ps.tile([128, N], F32, tag="fn", bufs=2, name="op_ps")
                        for dc in range(FC):
                            nc.tensor.matmul(op_ps, ow_t[:, dc, fc * 128:(fc + 1) * 128],
                                             ou[:, dc, :],
                                             start=(dc == 0), stop=(dc == FC - 1))
                        nc.vector.tensor_add(hgt[:, fc, :], hgt[:, fc, :], op_ps)

                    # ---- LN1 ----
                    _layernorm(nc, wk, ps, flags, bias_sb, bcol, "ln1", l,
                               hgt, invn, ones128r, onescol)

                    # ---- FFN ----
                    x1 = wk.tile([128, FFC, N], BF16, bufs=1, name="x1")
                    for ffc in range(FFC):
                        f1_ps = ps.tile([128, N], F32, tag="fn", bufs=2, name="f1_ps")
                        for kc in range(KC):
                            nc.tensor.matmul(f1_ps, l1_t[:, kc, ffc * 128:(ffc + 1) * 128],
                                             hgt[:, kc, :],
                                             start=(kc == 0), stop=(kc == KC - 1))
                        if flags["l1b"]:
                            nc.scalar.activation(x1[:, ffc, :], f1_ps, AF.Relu,
                                                 bias=bias_ap("l1b", l, ffc))
                        elif ffc % 2 == 0:
                            nc.vector.tensor_scalar_max(x1[:, ffc, :], f1_ps, 0.0)
                        else:
                            nc.scalar.activation(x1[:, ffc, :], f1_ps, AF.Relu)
                    for fc in range(FC):
                        f2_ps = ps.tile([128, N], F32, tag="fn", bufs=2, name="f2_ps")
                        for kc in range(FFC):
                            nc.tensor.matmul(f2_ps, l2_t[:, kc, fc * 128:(fc + 1) * 128],
                                             x1[:, kc, :],
                                             start=(kc == 0), stop=(kc == FFC - 1))
                        nc.vector.tensor_add(hgt[:, fc, :], hgt[:, fc, :], f2_ps)

                    # ---- LN2 ----
                    _layernorm(nc, wk, ps, flags, bias_sb, bcol, "ln2", l,
                               hgt, invn, ones128r, onescol)

            # ---- classifier ----
            for g in range(G):
                xcls = wk.tile([128, FC, N], BF16, bufs=1, name="xcls")
                for fc in range(FC):
                    c1_ps = ps.tile([128, N], F32, tag="qkv", bufs=1, name="c1_ps")
                    for kc in range(FC):
                        nc.tensor.matmul(c1_ps, c1T_sb[:, kc, fc * 128:(fc + 1) * 128],
                                         hg[g][:, kc, :],
                                         start=(kc == 0), stop=(kc == FC - 1))
                    if flags["c1b"]:
                        nc.scalar.activation(xcls[:, fc, :], c1_ps, AF.Relu,
                                             bias=bias_sb[:, bcol["c1b"] + fc:bcol["c1b"] + fc + 1])
                    else:
                        nc.scalar.activation(xcls[:, fc, :], c1_ps, AF.Relu)
                # transposed final matmul: out[c, t] = sum_f W2[c,f] xcls[f,t]
                c2_ps = ps.tile([128, N], F32, tag="qkv", bufs=1, name="c2_ps")
                for ic in range(FC):
                    nc.tensor.matmul(c2_ps[0:C, :], c2T_sb[:, ic, :], xcls[:, ic, :],
                                     start=(ic == 0), stop=(ic == FC - 1 and not flags["c2b"]))
                if flags["c2b"]:
                    raise NotImplementedError("non-zero cls2_b not supported")
                outsb = wk.tile([C, N], F32, bufs=2, name="outsb")
                nc.scalar.activation(outsb, c2_ps[0:C, :], AF.Copy)
                nc.sync.dma_start(out=d_out[g], in_=outsb)

    nc.compile()
    return nc


def _layernorm(nc, wk, ps, flags, bias_sb, bcol, which, l,
               hgt, invn, ones128r, onescol):
    """LayerNorm over features (partitions) for graph g, in place on h_sb.

    stats: col-tiled concurrent matmul pairs accumulate mean (partition 0)
    and mean-square (partition 32) rows of one psum bank; row math on [1,N];
    rank-1 bf16 broadcasts; quad-batched DVE apply.
    """
    stats = ps.tile([128, N], F32, tag="ln", bufs=1, name="stats")
    # u^2 in two halves so sq matmuls can start early
    u2 = wk.tile([128, 2, N], BF16, bufs=1, name="u2")
    u2b = wk.tile([128, 2, N], BF16, bufs=1, name="u2b")
    nc.scalar.activation(u2, hgt[:, 0:2, :], AF.Square)
    nc.scalar.activation(u2b, hgt[:, 2:4, :], AF.Square)
    for fc in range(FC):
        nc.tensor.matmul(stats[0:1, :], invn, hgt[:, fc, :],
                         start=(fc == 0), stop=(fc == FC - 1),
                         tile_position=(0, 0))
        usrc = u2[:, fc, :] if fc < 2 else u2b[:, fc - 2, :]
        nc.tensor.matmul(stats[32:33, :], invn, usrc,
                         start=(fc == 0), stop=(fc == FC - 1),
                         tile_position=(0, 32))
    # row math
    m_bf = wk.tile([1, N], BF16, bufs=1, name="m_bf")
    nc.scalar.activation(m_bf, stats[0:1, :], AF.Copy)
    sq_f = wk.tile([1, N], F32, bufs=1, name="sq_f")
    nc.scalar.activation(sq_f, stats[32:33, :], AF.Copy)
    m2_f = wk.tile([1, N], F32, bufs=1, name="m2_f")
    nc.vector.tensor_mul(m2_f, m_bf, m_bf)
    var_f = wk.tile([1, N], F32, bufs=1, name="var_f")
    nc.vector.scalar_tensor_tensor(out=var_f, in0=sq_f, scalar=1e-5, in1=m2_f,
                                   op0=ALU.add, op1=ALU.subtract)
    rv_f = wk.tile([1, N], F32, bufs=1, name="rv_f")
    nc.vector.reciprocal_approx_fast(out=rv_f, in_=var_f)
    rstd_bf = wk.tile([1, N], BF16, bufs=1, name="rstd_bf")
    nc.scalar.activation(rstd_bf, rv_f, AF.Sqrt)
    # broadcast m and rstd to [128, N] one at a time through a single psum
    # bank, staging each in SBUF bf16 (PSUM-sourced DVE runs at half rate)
    s_on = flags[which + "s"]
    b_on = flags[which + "b"]
    lnb = ps.tile([128, N], F32, tag="ln", bufs=1, name="lnb")
    nc.tensor.matmul(lnb, ones128r, m_bf, start=True, stop=True)
    mb_sb = wk.tile([128, N], BF16, bufs=1, name="mb_sb")
    nc.scalar.activation(mb_sb, lnb, AF.Copy)
    lnb2 = ps.tile([128, N], F32, tag="ln", bufs=1, name="lnb2")
    nc.tensor.matmul(lnb2, ones128r, rstd_bf, start=True, stop=True)
    rb_sb = wk.tile([128, N], BF16, bufs=1, name="rb_sb")
    nc.scalar.activation(rb_sb, lnb2, AF.Copy)
    hq = hgt[:, :, :]
    mb_bc = bass.AP(tensor=mb_sb.tensor, offset=mb_sb.offset,
                    ap=[mb_sb.ap[0], [0, FC]] + list(mb_sb.ap[1:]))
    rb_bc = bass.AP(tensor=rb_sb.tensor, offset=rb_sb.offset,
                    ap=[rb_sb.ap[0], [0, FC]] + list(rb_sb.ap[1:]))
    nc.vector.tensor_sub(hq, hq, mb_bc)
    if not s_on and not b_on:
        nc.vector.tensor_mul(hq, hq, rb_bc)
    else:
        nc.vector.tensor_mul(hq, hq, rb_bc)
        for fc in range(FC):
            sc = bias_sb[:, bcol[which + "s"] + l * 4 + fc:bcol[which + "s"] + l * 4 + fc + 1] \
                if s_on else 1.0
            bi = bias_sb[:, bcol[which + "b"] + l * 4 + fc:bcol[which + "b"] + l * 4 + fc + 1] \
                if b_on else 0.0
            nc.scalar.activation(h_sb[:, fc, gsl], h_sb[:, fc, gsl], AF.Copy,
                                 bias=bi, scale=sc)


_CACHE = {}


def _prep_inputs(inputs):
    """Host-side preprocessing -> (flags, per-core in_maps)."""
    x = np.asarray(inputs["x"], np.float32)
    pe = np.asarray(inputs["pe"], np.float32)
    degree = np.asarray(inputs["degree"], np.float32)
    W_emb = np.asarray(inputs["W_emb"], np.float32)
    qkv_w = np.asarray(inputs["qkv_w"], np.float32)
    qkv_b = np.asarray(inputs["qkv_b"], np.float32)
    out_w = np.asarray(inputs["out_w"], np.float32)
    out_b = np.asarray(inputs["out_b"], np.float32)
    lin1_w = np.asarray(inputs["lin1_w"], np.float32)
    lin1_b = np.asarray(inputs["lin1_b"], np.float32)
    lin2_w = np.asarray(inputs["lin2_w"], np.float32)
    lin2_b = np.asarray(inputs["lin2_b"], np.float32)
    ln1_s = np.asarray(inputs["ln1_s"], np.float32)
    ln1_b = np.asarray(inputs["ln1_b"], np.float32)
    ln2_s = np.asarray(inputs["ln2_s"], np.float32)
    ln2_b = np.asarray(inputs["ln2_b"], np.float32)
    cls1_w = np.asarray(inputs["cls1_w"], np.float32)
    cls1_b = np.asarray(inputs["cls1_b"], np.float32)
    cls2_w = np.asarray(inputs["cls2_w"], np.float32)
    cls2_b = np.asarray(inputs["cls2_b"], np.float32)

    flags = {
        "qkvb": bool(np.any(qkv_b[:, :2 * D])),
        "vb": bool(np.any(qkv_b[:, 2 * D:])),
        "ob": bool(np.any(out_b)),
        "l1b": bool(np.any(lin1_b)),
        "l2b": bool(np.any(lin2_b)),
        "c1b": bool(np.any(cls1_b)),
        "c2b": bool(np.any(cls2_b)),
        "ln1s": bool(np.any(ln1_s != 1.0)),
        "ln1b": bool(np.any(ln1_b)),
        "ln2s": bool(np.any(ln2_s != 1.0)),
        "ln2b": bool(np.any(ln2_b)),
    }
    if flags["ob"] or flags["l2b"] or flags["c2b"]:
        raise NotImplementedError("non-zero out_b/lin2_b/cls2_b not supported")

    # bias blob layout
    cols = {}
    ncols = 0

    def add(name, width):
        nonlocal ncols
        cols[name] = ncols
        ncols += width

    blob_parts = []
    if flags["qkvb"]:
        add("qkvb", L * 8)
        qb = qkv_b[:, :2 * D].reshape(L, 8, 128)
        qb = qb.copy()
        qb[:, :4, :] *= 0.125  # q-bias carries the 1/sqrt(dh) fold
        blob_parts.append(qb.transpose(2, 0, 1).reshape(128, L * 8))
    if flags["l1b"]:
        add("l1b", L * 16)
        blob_parts.append(lin1_b.reshape(L, 16, 128).transpose(2, 0, 1).reshape(128, L * 16))
    if flags["c1b"]:
        add("c1b", 4)
        blob_parts.append(cls1_b.reshape(4, 128).T)
    for nm, arr in (("ln1s", ln1_s), ("ln1b", ln1_b), ("ln2s", ln2_s), ("ln2b", ln2_b)):
        if flags[nm]:
            add(nm, L * 4)
            blob_parts.append(arr.reshape(L, 4, 128).transpose(2, 0, 1).reshape(128, L * 4))
    flags["bias_cols"] = cols
    flags["nbias_cols"] = ncols
    blob = np.concatenate(blob_parts, axis=1).astype(np.float32) if blob_parts else None

    qkvT = np.ascontiguousarray(qkv_w.transpose(0, 2, 1)).copy()
    qkvT[:, :, :D] *= 0.125
    shared = {
        "wembT": np.ascontiguousarray(W_emb.T).astype(bf),
        "qkvT": qkvT.astype(bf),
        "owT": np.ascontiguousarray(out_w.transpose(0, 2, 1)).astype(bf),
        "l1T": np.ascontiguousarray(lin1_w.transpose(0, 2, 1)).astype(bf),
        "l2T": np.ascontiguousarray(lin2_w.transpose(0, 2, 1)).astype(bf),
        "c1T": np.ascontiguousarray(cls1_w.T).astype(bf),
        "c2T": np.ascontiguousarray(cls2_w.T).astype(bf),
    }
    if blob is not None:
        shared["biasblob"] = blob
    if flags["vb"]:
        shared["vbias"] = qkv_b[:, 2 * D:].reshape(L, 1, D).astype(bf)

    peb = (pe + 1e-6).transpose(0, 2, 1)
    xT = x.transpose(0, 2, 1)
    in_maps = []
    for c in range(NCORES):
        gs = slice(c * G, (c + 1) * G)
        im = dict(shared)
        im["xT"] = np.ascontiguousarray(xT[gs]).astype(bf)
        im["peT"] = np.ascontiguousarray(peb[gs]).astype(bf)
        im["deg"] = np.ascontiguousarray(degree[gs]).astype(np.float32)
        in_maps.append(im)
    return flags, in_maps


def _gather_out(res):
    # res core outputs are [G, C, N]; -> [B*N, C]
    outs = []
    for r in res.results:
        o = r["out"]                       # [G, C, N]
        outs.append(np.ascontiguousarray(o.transpose(0, 2, 1)).reshape(G * N, C))
    return np.concatenate(outs, axis=0).astype(np.float32)


def kernel(**inputs):
    flags, in_maps = _prep_inputs(inputs)
    key = tuple(sorted((k, v) for k, v in flags.items() if k not in ("bias_cols",)
                       and not isinstance(v, dict)))
    if key not in _CACHE:
        _CACHE[key] = _build(flags)
    nc = _CACHE[key]
    res = bass_utils.run_bass_kernel_spmd(nc, in_maps, list(range(NCORES)))
    return _gather_out(res)


def run_traced(inputs, tmpdir="/tmp/ntff_out"):
    """For test.py: run with NTFF tracing, return (output, results)."""
    import shutil
    flags, in_maps = _prep_inputs(inputs)
    nc = _build(flags)
    if tmpdir:
        shutil.rmtree(tmpdir, ignore_errors=True)
        os.makedirs(tmpdir, exist_ok=True)
    res = bass_utils.run_bass_kernel_spmd(nc, in_maps, list(range(NCORES)),
                                          trace=True, tmpdir=tmpdir)
    return _gather_out(res), res
